# revision 60
# baseline (speedup 1.0000x reference)
"""Trainium2 Bass kernel for the HPM gaussian-ray read problem.

out[b,c] = sum_n exp(-r2[n,b]/(2*sigma^2)) * exp(-max(t[n,b],0)/tau) * mem[n,c]

over the flattened 128^3 grid (N = 2,097,152), B=32 rays, C=16 channels.

Sparsity: with sigma=0.5 and tau=2 each ray's Gaussian tube touches only a
thin set of (gx,gy) grid columns ("chunks"); only ~7700 of the 524288
(chunk, ray) pairs have a kern z-sum above S_THRESH (provable kmax upper
bound prunes the candidate set; exact z-sums refine it). Pair z-supports
are tiny (p99 = 8 of 128 z values), so each processing round is bound to a
fixed 64-z window ("flavor", start in {0, 32, 64}); every pair support
(<= 22 wide) fits some window. Host packs kept pairs into per-core groups
of its round's flavor: a group holds up to 8 chunks (one PE stationary mem
tile [64 z-window, 8*16 (slot,c)]) and up to 16 pair-columns.

Device kernel, per two-round slab (rounds stacked along SBUF partitions:
round A in partitions 0-63, round B in 64-127, exploiting PE-array tile
positions for base-64 matmul operands):
    PE mm1x2 : per-flavor 11-row bf16 basis (64 window z's) x bf16 split
               coefficients -> W columns, [128, CPR] fp32, one PSUM bank
               (the two mm1s run in different PE column tiles); only the
               first PG groups carry (W0, W1) branch pairs — ~99% of pairs
               never straddle t=0 inside their z-support and use a single
               branch column
    DVE min  : W = min(W0, W1) pairwise reduce on the paired section
    ACT exp  : kern = exp(W) -> bf16  (wm section + direct-from-PSUM rest)
    PE mm2   : per group g: psO[:, 16 cols] = memwin_g^T @ kern[z-half,
               g-slice] (each pair-column yields the 16 channel sums in
               the rows of its chunk's slot; host extracts and
               scatter-adds by ray)
    DVE copy : psO -> fp16 SBUF, one DMA out per slab
The ~200 GB/s per-core HBM->SBUF path (single sync-engine HW-DGE queue;
rate scales with descriptor partition-row count) is the bottleneck, so
mem slabs are 128-row transfers carrying only the 64-z windows
(~1.3 MB/core) and outputs are fp16.

Sharding: kept chunks are interleaved across the 8 cores (a shard of the
flattened N axis per the hint); host sums the per-pair partials into [B,C].
"""

import numpy as np

SIGMA = 0.5
TAU = 2.0
NCORES = 8
D = 128           # grid edge
B = 32            # rays
C = 16            # channels
KROWS = 11        # split-bf16 basis rows
NCHUNK = D * D    # 16384 (gx,gy) columns, 128 z's each
S_THRESH = 1e-2   # drop (chunk, ray) pairs whose z-sum of kern is below this
ZEPS = 1e-4       # per-pair kern mass allowed outside its z-support window

SLOTS = 8         # chunks per group (PE stationary tile)
MP = 16           # pair-column budget per group
PG = 1            # paired (two-branch) groups per round; the rest are
                  # single-branch (only ~1% of pairs straddle t=0)
BTOL = 1e-3       # branch-sum tolerance for single-branch classification
NG = 84           # groups per core
NROUND = 6                         # pipeline rounds (16 groups each)
GRP_PER_ROUND = NG // NROUND
ZWIN = 64                          # z-window length per round
NFLAV = 3                          # window starts: 0, 32, 64
FLAV_OF_ROUND = (0, 1, 2, 0, 1, 2)

_BASS_CACHE = {}


def _build_nc():
    """Build the (per-core identical) Bass program."""
    from contextlib import ExitStack
    import concourse.bacc as bacc
    import concourse.mybir as mybir
    from concourse.tile import TileContext

    f32 = mybir.dt.float32
    f16 = mybir.dt.float16
    bf16 = mybir.dt.bfloat16
    PAIRS_RND = GRP_PER_ROUND * MP                 # pair-columns per round
    RND_COLS = GRP_PER_ROUND * SLOTS * C           # mem cols per round
    CPR = PG * MP * 2 + (GRP_PER_ROUND - PG) * MP  # coef cols per round
    ZCOLS = NFLAV * ZWIN
    nc = bacc.Bacc()
    # zaug and coef share one 11-partition tensor (one DMA)
    cz_d = nc.dram_tensor("cz", [KROWS, ZCOLS + NROUND * CPR], bf16,
                          kind="ExternalInput")
    # slab sl: partitions 0-63 = round 2sl's z-window, 64-127 = round 2sl+1
    mem_d = nc.dram_tensor("mem", [NROUND // 2, 2 * ZWIN, RND_COLS],
                           bf16, kind="ExternalInput")
    out_d = nc.dram_tensor("out", [NROUND // 2, D, 2 * PAIRS_RND], f16,
                           kind="ExternalOutput")

    with TileContext(nc) as tc:
        with ExitStack() as ctx:
            singles = ctx.enter_context(tc.tile_pool(name="singles", bufs=1))
            mempool = ctx.enter_context(tc.tile_pool(name="memp", bufs=3))
            wpool = ctx.enter_context(tc.tile_pool(name="wp", bufs=2))
            kpool = ctx.enter_context(tc.tile_pool(name="kp", bufs=2))
            outpool = ctx.enter_context(tc.tile_pool(name="outp", bufs=2))
            pswpool = ctx.enter_context(tc.tile_pool(name="psw", bufs=2, space="PSUM"))
            psopool = ctx.enter_context(tc.tile_pool(name="pso", bufs=2, space="PSUM"))

            # everything on the sync HW-DGE queue (the scalar queue steals
            # from the same ~190GB/s pipe); compute-critical loads first
            cz = singles.tile([KROWS, ZCOLS + NROUND * CPR], bf16)
            nc.sync.dma_start(out=cz[:], in_=cz_d[:, :])
            zaug = cz[:, 0:ZCOLS]
            coefall = cz[:, ZCOLS:]

            bigmems = []
            for sl in range(NROUND // 2):
                bigmem = mempool.tile([2 * ZWIN, RND_COLS], bf16)
                if sl == NROUND // 2 - 1:
                    # split the last slab so its first groups' mm2 wave
                    # overlaps the trailing half of the transfer
                    hc = RND_COLS // 2
                    nc.sync.dma_start(out=bigmem[:, 0:hc],
                                      in_=mem_d[sl, :, 0:hc])
                    nc.sync.dma_start(out=bigmem[:, hc:],
                                      in_=mem_d[sl, :, hc:])
                else:
                    nc.sync.dma_start(out=bigmem[:], in_=mem_d[sl])
                bigmems.append(bigmem)

            for sl in range(NROUND // 2):
                bigmem = bigmems[sl]
                # both rounds' W columns into one PSUM bank, stacked by
                # partition half (tile_position rows/cols from AP bases)
                psW = pswpool.tile([2 * ZWIN, CPR], f32)
                for h in range(2):
                    rnd = 2 * sl + h
                    zs = FLAV_OF_ROUND[rnd] * ZWIN
                    cs = rnd * CPR
                    nc.tensor.matmul(psW[h * ZWIN:(h + 1) * ZWIN, :],
                                     zaug[:, zs:zs + ZWIN],
                                     coefall[:, cs:cs + CPR],
                                     start=True, stop=True)

                # paired section: W = min(W0, W1); single section: W direct
                wm = wpool.tile([2 * ZWIN, PG * MP], f32)
                pw = psW[:, 0:2 * PG * MP].rearrange("p (jb s) -> p jb s", s=2)
                nc.vector.tensor_reduce(
                    wm[:], pw, axis=mybir.AxisListType.X,
                    op=mybir.AluOpType.min)

                kern = kpool.tile([2 * ZWIN, PAIRS_RND], bf16)
                nc.scalar.activation(kern[:, 0:PG * MP], wm[:],
                                     mybir.ActivationFunctionType.Exp)
                nc.scalar.activation(kern[:, PG * MP:], psW[:, 2 * PG * MP:],
                                     mybir.ActivationFunctionType.Exp)

                outsb = outpool.tile([D, 2 * PAIRS_RND], f16)
                if sl == NROUND // 2 - 1:
                    # last slab: group-major emission so both halves' leading
                    # groups run during the trailing half-DMA
                    psO_a = psopool.tile([D, PAIRS_RND], f32)
                    psO_b = psopool.tile([D, PAIRS_RND], f32)
                    psOs = [psO_a, psO_b]
                    for g in range(GRP_PER_ROUND):
                        for h in range(2):
                            z0, z1 = h * ZWIN, (h + 1) * ZWIN
                            nc.tensor.matmul(
                                psOs[h][:, g * MP:(g + 1) * MP],
                                bigmem[z0:z1, g * 128:(g + 1) * 128],
                                kern[z0:z1, g * MP:(g + 1) * MP],
                                start=True, stop=True)
                    for h in range(2):
                        os = h * PAIRS_RND
                        nc.vector.tensor_copy(
                            out=outsb[:, os:os + PAIRS_RND], in_=psOs[h][:])
                else:
                    for h in range(2):
                        z0, z1 = h * ZWIN, (h + 1) * ZWIN
                        psO = psopool.tile([D, PAIRS_RND], f32)
                        for g in range(GRP_PER_ROUND):
                            nc.tensor.matmul(psO[:, g * MP:(g + 1) * MP],
                                             bigmem[z0:z1, g * 128:(g + 1) * 128],
                                             kern[z0:z1, g * MP:(g + 1) * MP],
                                             start=True, stop=True)
                        os = h * PAIRS_RND
                        nc.vector.tensor_copy(out=outsb[:, os:os + PAIRS_RND],
                                              in_=psO[:])
                nc.sync.dma_start(out=out_d[sl], in_=outsb[:])

    nc.compile()
    return nc


def _get_nc():
    if "nc" not in _BASS_CACHE:
        _BASS_CACHE["nc"] = _build_nc()
    return _BASS_CACHE["nc"]


def _bf16(x):
    import ml_dtypes
    return x.astype(ml_dtypes.bfloat16)


def _split3(x):
    """f64 -> three bf16 parts summing to ~24 mantissa bits of x."""
    x0 = _bf16(x).astype(np.float64)
    x1 = _bf16(x - x0).astype(np.float64)
    x2 = _bf16(x - x0 - x1).astype(np.float64)
    return x0, x1, x2


def _host_coeffs(ray_origin, ray_dir):
    """Quadratic coefficients of W0/W1 in u = z-64, in f64."""
    o = ray_origin.astype(np.float64)
    d = ray_dir.astype(np.float64)
    d2 = (d * d).sum(-1)
    kap = 2.0 - d2
    od = (o * d).sum(-1)
    g = np.arange(D, dtype=np.float64)
    gxy_x = np.repeat(g, D)
    gxy_y = np.tile(g, D)
    c1 = 1.0 / (2 * SIGMA ** 2)
    c3 = 1.0 / TAU
    alpha = gxy_x[:, None] * d[None, :, 0] + gxy_y[:, None] * d[None, :, 1] - od[None, :]
    t64 = 64.0 * d[None, :, 2] + alpha                      # [NCHUNK, B]
    e = 64.0 - o[:, 2]                                      # [B]
    gamma = (gxy_x[:, None] - o[None, :, 0]) ** 2 + (gxy_y[:, None] - o[None, :, 1]) ** 2
    A0 = np.broadcast_to((-c1 + c1 * kap * d[:, 2] ** 2)[None, :], t64.shape)
    B0 = -2 * c1 * e[None, :] + 2 * c1 * kap[None, :] * d[None, :, 2] * t64
    C0 = -c1 * (gamma + e[None, :] ** 2) + c1 * kap[None, :] * t64 ** 2
    B1 = B0 - c3 * d[None, :, 2]
    C1 = C0 - c3 * t64
    return A0, B0, C0, B1, C1


def _pack_cols(Aq, Bq, Cq):
    """[...] f64 quadratic -> [11, ...] bf16 split rows.
    Row order: [C0,B0,Ah0,Al0, C1,B1,Ah1,Al1, C2,B2,Ah2]."""
    C_0, C_1, C_2 = _split3(Cq)
    B_0, B_1, B_2 = _split3(Bq)
    A_0, A_1, A_2 = _split3(Aq)
    rows = [C_0, B_0, A_0, A_0, C_1, B_1, A_1, A_1, C_2, B_2, A_2]
    return np.stack([_bf16(r) for r in rows])


def _zaug_rows():
    """Per-flavor basis columns: [11, NFLAV * ZWIN] bf16."""
    cols = []
    for f in range(NFLAV):
        z0 = f * 32
        u = np.arange(z0, z0 + ZWIN, dtype=np.float64) - 64.0
        u2 = u * u
        uh = _bf16(u2).astype(np.float64)
        ul = u2 - uh
        one = np.ones_like(u)
        rows = [one, u, uh, ul, one, u, uh, ul, one, u, uh]
        cols.append(np.stack([_bf16(r) for r in rows]))
    return np.concatenate(cols, axis=1)


def _select_pairs(A0, B0, C0, B1, C1):
    """Kept (chunk, ray) pairs with exact z-sums >= S_THRESH and their
    z-supports [lo, hi). Returns (chunk, ray, lo, hi) sorted by chunk."""
    u = np.arange(D, dtype=np.float64) - 64.0

    def grid_max(Bq, Cq):
        us = np.round(np.clip(-Bq / (2 * A0), u[0], u[-1]))
        best = A0 * us * us + Bq * us + Cq
        for dd in (-1.0, 1.0):
            u2 = np.clip(us + dd, u[0], u[-1])
            best = np.maximum(best, A0 * u2 * u2 + Bq * u2 + Cq)
        return best

    wmax = np.minimum(grid_max(B0, C0), grid_max(B1, C1))
    kmax = np.exp(np.maximum(wmax, -745.0))
    # S <= 128 * kmax, so this candidate set provably covers {S >= thresh}
    ci, ri = np.nonzero(kmax >= S_THRESH / 256.0)
    a, b0, c0 = A0[ci, ri, None], B0[ci, ri, None], C0[ci, ri, None]
    b1, c1 = B1[ci, ri, None], C1[ci, ri, None]
    W = np.minimum(a * u * u + b0 * u + c0, a * u * u + b1 * u + c1)
    kern = np.exp(np.maximum(W, -745.0))
    S = kern.sum(-1)
    keep = S >= S_THRESH
    ci, ri, kern = ci[keep], ri[keep], kern[keep]
    csum = np.cumsum(kern, -1)
    tot = csum[:, -1:]
    lo = (csum < ZEPS).sum(-1)
    hi = D - ((tot - csum) < ZEPS).sum(-1)
    hi = np.maximum(hi, lo + 1)
    # clamp ultra-wide supports (none expected) to their heaviest window
    wide = (hi - lo) > ZWIN
    if wide.any():
        zc = np.argmax(kern[wide], -1)
        lo[wide] = np.clip(zc - ZWIN // 2, 0, D - ZWIN)
        hi[wide] = lo[wide] + ZWIN
    # branch type: 0 = W0 alone suffices, 1 = W1 alone, 2 = straddler
    zz = np.arange(D)[None, :]
    msk = (zz >= lo[:, None]) & (zz < hi[:, None])
    a, b0, c0 = A0[ci, ri, None], B0[ci, ri, None], C0[ci, ri, None]
    b1, c1 = B1[ci, ri, None], C1[ci, ri, None]
    S0 = (np.exp(np.maximum(a * u * u + b0 * u + c0, -745.0)) * msk).sum(-1)
    S1 = (np.exp(np.maximum(a * u * u + b1 * u + c1, -745.0)) * msk).sum(-1)
    Sm = (kern * msk).sum(-1)
    bt = np.full(len(ci), 2, np.int64)
    bt[(S1 - Sm) <= BTOL] = 1
    bt[(S0 - Sm) <= BTOL] = 0
    return ci, ri, lo, hi, bt


def _flavors_of(lo, hi):
    """Bitmask of window flavors whose [32f, 32f+ZWIN) covers [lo, hi)."""
    m = 0
    for f in range(NFLAV):
        z0 = 32 * f
        if z0 <= lo and hi <= z0 + ZWIN:
            m |= 1 << f
    return m


def _pack_core(items):
    """items: list of (chunk, [(ray, lo, hi), ...]). Split per chunk into
    flavor-subsets (keeping each subset's full allowed-flavor mask), assign
    subsets to flavors balancing chunk-slot load, then bin-pack each flavor
    into groups of <= SLOTS chunks / <= MP pairs laid out into its fixed
    rounds. Returns (slot_chunk [NG, SLOTS], pr_slot/pr_ray/pr_valid)."""
    subsets = []
    for chunk, rays in items:
        rem = [((r, bt), _flavors_of(lo, hi)) for (r, lo, hi, bt) in rays]
        while rem:
            # pick the flavor covering the most remaining rays
            cnt = [sum(1 for _, m in rem if m >> f & 1) for f in range(NFLAV)]
            f = int(np.argmax(cnt))
            taken = [(rb, m) for rb, m in rem if m >> f & 1]
            rem = [(rb, m) for rb, m in rem if not (m >> f & 1)]
            mk = (1 << NFLAV) - 1
            for _, m in taken:
                mk &= m
            subsets.append((chunk, [rb for rb, _ in taken], mk))
    # assign inflexible subsets first, then flexible ones to the lightest
    # flavor (chunk-slots are the binding capacity)
    subsets.sort(key=lambda s: bin(s[2]).count("1"))
    loads = [0] * NFLAV
    flav_items = [[] for _ in range(NFLAV)]
    for chunk, rays, mk in subsets:
        cand = [f for f in range(NFLAV) if mk >> f & 1]
        f = min(cand, key=lambda f: loads[f])
        flav_items[f].append((chunk, rays))
        loads[f] += 1

    slot_chunk = np.full((NG, SLOTS), -1, np.int64)
    pr_slot = np.zeros((NG, MP), np.int64)
    pr_ray = np.zeros((NG, MP), np.int64)
    pr_btype = np.zeros((NG, MP), np.int64)
    pr_valid = np.zeros((NG, MP), bool)
    rounds_of_flav = [[r for r in range(NROUND) if FLAV_OF_ROUND[r] == f]
                      for f in range(NFLAV)]
    for f in range(NFLAV):
        # paired group slots first; straddler-containing items lead so
        # every straddler pair lands in a paired group
        slots_avail = (
            [r * GRP_PER_ROUND + j for r in rounds_of_flav[f]
             for j in range(PG)] +
            [r * GRP_PER_ROUND + j for r in rounds_of_flav[f]
             for j in range(PG, GRP_PER_ROUND)])
        fitems = sorted(flav_items[f],
                        key=lambda it: -max(bt for _, bt in it[1]))
        gi = 0
        g = slots_avail[0]
        ns = 0
        npair = 0
        for chunk, rays in fitems:
            while len(rays) > 0:
                take = rays[:MP]
                if ns >= SLOTS or npair + len(take) > MP:
                    gi += 1
                    if gi >= len(slots_avail):
                        raise RuntimeError(
                            "group capacity exceeded; raise S_THRESH")
                    g = slots_avail[gi]
                    ns = 0
                    npair = 0
                slot_chunk[g, ns] = chunk
                for r, bt in take:
                    if bt == 2 and g % GRP_PER_ROUND >= PG:
                        raise RuntimeError("straddler outside paired group")
                    pr_slot[g, npair] = ns
                    pr_ray[g, npair] = r
                    pr_btype[g, npair] = bt
                    pr_valid[g, npair] = True
                    npair += 1
                ns += 1
                rays = rays[MP:]
    return slot_chunk, pr_slot, pr_ray, pr_btype, pr_valid


def _prep_inputs(ray_origin, ray_dir, memory):
    import ml_dtypes
    A0, B0, C0, B1, C1 = _host_coeffs(ray_origin, ray_dir)
    w0 = _pack_cols(A0, B0, C0)          # [11, NCHUNK, B]
    w1 = _pack_cols(A0, B1, C1)
    zaug = _zaug_rows()

    ci, ri, lo, hi, bt = _select_pairs(A0, B0, C0, B1, C1)
    order = np.argsort(ci, kind="stable")
    ci, ri, lo, hi, bt = ci[order], ri[order], lo[order], hi[order], bt[order]
    uchunks, starts = np.unique(ci, return_index=True)
    starts = list(starts) + [len(ci)]
    items = [(uchunks[j],
              [(ri[t], lo[t], hi[t], bt[t])
               for t in range(starts[j], starts[j + 1])])
             for j in range(len(uchunks))]

    mem = np.ascontiguousarray(memory, dtype=np.float32).reshape(NCHUNK, D, C)
    mem_bf = mem.astype(ml_dtypes.bfloat16)
    in_maps = []
    extract = []
    CPR = PG * MP * 2 + (GRP_PER_ROUND - PG) * MP
    for k in range(NCORES):
        slot_chunk, pr_slot, pr_ray, pr_btype, pr_valid = _pack_core(
            items[k::NCORES])
        safe = np.maximum(slot_chunk, 0)
        # z-windowed mem gather: group g uses window of its round's flavor
        z0g = np.array([32 * FLAV_OF_ROUND[g // GRP_PER_ROUND]
                        for g in range(NG)])
        zidx = z0g[:, None] + np.arange(ZWIN)[None, :]       # [NG, ZWIN]
        mk = mem_bf[safe[:, :, None], zidx[:, None, :]]      # [NG, SLOTS, ZWIN, C]
        mk[slot_chunk < 0] = 0
        # [NROUND // 2, 2 * ZWIN, GRP_PER_ROUND * SLOTS * C]: two-round
        # slabs stacked along the partition axis
        mk = np.ascontiguousarray(
            mk.reshape(NROUND // 2, 2, GRP_PER_ROUND, SLOTS, ZWIN, C)
            .transpose(0, 1, 4, 2, 3, 5)).reshape(NROUND // 2, 2 * ZWIN, -1)

        # coef columns: paired groups get (W0, W1) per pair (clean pairs
        # duplicate their active branch); single groups get one branch
        pc = np.maximum(slot_chunk, 0)[
            np.arange(NG)[:, None], np.maximum(pr_slot, 0)]   # chunk per pair
        w0c = w0[:, pc, pr_ray]                    # [11, NG, MP]
        w1c = w1[:, pc, pr_ray]
        wb = np.where((pr_btype == 1)[None], w1c, w0c)
        pa = np.where((pr_btype == 2)[None], w0c, wb)
        pb = np.where((pr_btype == 2)[None], w1c, wb)
        for arr in (wb, pa, pb):
            arr[:, ~pr_valid] = 0
            arr[0, ~pr_valid] = -30000.0           # kern = exp(-30000) = 0
        gidx = np.arange(NG).reshape(NROUND, GRP_PER_ROUND)
        paired = np.stack([pa[:, gidx[:, :PG]], pb[:, gidx[:, :PG]]],
                          axis=-1).reshape(KROWS, NROUND, PG * MP * 2)
        single = wb[:, gidx[:, PG:]].reshape(KROWS, NROUND, -1)
        ck = np.concatenate([paired, single], axis=2).reshape(KROWS, -1)
        cz = np.ascontiguousarray(np.concatenate([zaug, ck], axis=1))
        in_maps.append({"cz": cz, "mem": mk})
        extract.append((pr_slot, pr_ray, pr_valid))
    return in_maps, extract


def _extract(results, extract):
    out = np.zeros((B, C), np.float64)
    for res, (pr_slot, pr_ray, pr_valid) in zip(results, extract):
        blocks = res["out"].astype(np.float64)  # [NROUND//2, 128, 2*PAIRS_RND]
        arr = blocks.reshape(NROUND // 2, D, 2, GRP_PER_ROUND, MP)
        arr = arr.transpose(0, 2, 3, 1, 4).reshape(NG, D, MP)
        # val[g, p, c] = arr[g, 16 * pr_slot[g, p] + c, p]
        arrT = arr.transpose(0, 2, 1)              # [NG, MP, 128]
        row = (C * pr_slot)[:, :, None] + np.arange(C)[None, None, :]
        val = np.take_along_axis(arrT, row, axis=2)   # [NG, MP, C]
        np.add.at(out, pr_ray[pr_valid], val[pr_valid])
    return np.ascontiguousarray(out).astype(np.float32)   # [B, C]


def run_kernel(ray_origin, ray_dir, memory, trace=False, **run_kwargs):
    """Run on 8 NeuronCores; returns ([B,C] output, BassKernelResults)."""
    from concourse.bass_utils import run_bass_kernel_spmd
    nc = _get_nc()
    in_maps, extract = _prep_inputs(ray_origin, ray_dir, memory)
    br = run_bass_kernel_spmd(nc, in_maps, core_ids=list(range(NCORES)),
                              trace=trace, **run_kwargs)
    return _extract(br.results, extract), br


def kernel(ray_origin, ray_dir, memory):
    out, _ = run_kernel(np.asarray(ray_origin), np.asarray(ray_dir),
                        np.asarray(memory))
    return out


# revision 63
# speedup vs baseline: 1.2089x; 1.2089x over previous
"""Trainium2 Bass kernel for the HPM gaussian-ray read problem.

out[b,c] = sum_n exp(-r2[n,b]/(2*sigma^2)) * exp(-max(t[n,b],0)/tau) * mem[n,c]

over the flattened 128^3 grid (N = 2,097,152), B=32 rays, C=16 channels.

Sparsity: with sigma=0.5 and tau=2 each ray's Gaussian tube touches only a
thin set of (gx,gy) grid columns ("chunks"); only ~7700 of the 524288
(chunk, ray) pairs have a kern z-sum above S_THRESH (provable kmax upper
bound prunes the candidate set; exact z-sums refine it). Pair z-supports
are tiny (p99 = 8 of 128 z values), so each processing round is bound to a
fixed 64-z window ("flavor", start in {0, 32, 64}); every pair support
(<= 22 wide) fits some window. Host packs kept pairs into per-core groups
of its round's flavor: a group holds up to 8 chunks (one PE stationary mem
tile [64 z-window, 8*16 (slot,c)]) and up to 16 pair-columns.

Device kernel, per two-round slab (rounds stacked along SBUF partitions:
round A in partitions 0-63, round B in 64-127, exploiting PE-array tile
positions for base-64 matmul operands):
    PE mm1x2 : per-flavor 11-row bf16 basis (64 window z's) x bf16 split
               coefficients -> W columns, [128, CPR] fp32, one PSUM bank
               (the two mm1s run in different PE column tiles); only the
               first PG groups carry (W0, W1) branch pairs — ~99% of pairs
               never straddle t=0 inside their z-support and use a single
               branch column
    DVE min  : W = min(W0, W1) pairwise reduce on the paired section
    ACT exp  : kern = exp(W) -> bf16  (wm section + direct-from-PSUM rest)
    PE mm2   : per group g: psO[:, 16 cols] = memwin_g^T @ kern[z-half,
               g-slice] (each pair-column yields the 16 channel sums in
               the rows of its chunk's slot; host extracts and
               scatter-adds by ray)
    DVE copy : psO -> fp16 SBUF, one DMA out per slab
The ~200 GB/s per-core HBM->SBUF path (single sync-engine HW-DGE queue;
rate scales with descriptor partition-row count) is the bottleneck, so
mem slabs are 128-row transfers carrying only the 64-z windows
(~1.3 MB/core) and outputs are fp16.

Sharding: kept chunks are interleaved across the 8 cores (a shard of the
flattened N axis per the hint); host sums the per-pair partials into [B,C].
"""

import numpy as np

SIGMA = 0.5
TAU = 2.0
NCORES = 8
D = 128           # grid edge
B = 32            # rays
C = 16            # channels
KROWS = 11        # split-bf16 basis rows
NCHUNK = D * D    # 16384 (gx,gy) columns, 128 z's each
S_THRESH = 1e-2   # drop (chunk, ray) pairs whose z-sum of kern is below this
ZEPS = 1e-4       # per-pair kern mass allowed outside its z-support window

SLOTS = 8         # chunks per group (PE stationary tile)
MP = 16           # pair-column budget per group
PG = 1            # paired (two-branch) groups per round; the rest are
                  # single-branch (only ~1% of pairs straddle t=0)
BTOL = 1e-3       # branch-sum tolerance for single-branch classification
NG = 84           # groups per core
NROUND = 6                         # pipeline rounds (16 groups each)
GRP_PER_ROUND = NG // NROUND
ZWIN = 64                          # z-window length per round
NFLAV = 3                          # window starts: 0, 32, 64
FLAV_OF_ROUND = (0, 1, 2, 0, 1, 2)

_BASS_CACHE = {}


def _build_nc():
    """Build the (per-core identical) Bass program."""
    from contextlib import ExitStack
    import concourse.bacc as bacc
    import concourse.mybir as mybir
    from concourse.tile import TileContext

    f32 = mybir.dt.float32
    f16 = mybir.dt.float16
    bf16 = mybir.dt.bfloat16
    PAIRS_RND = GRP_PER_ROUND * MP                 # pair-columns per round
    RND_COLS = GRP_PER_ROUND * SLOTS * C           # mem cols per round
    CPR = PG * MP * 2 + (GRP_PER_ROUND - PG) * MP  # coef cols per round
    ZCOLS = NFLAV * ZWIN
    nc = bacc.Bacc()
    # zaug and coef share one 11-partition tensor (one DMA)
    cz_d = nc.dram_tensor("cz", [KROWS, ZCOLS + NROUND * CPR], bf16,
                          kind="ExternalInput")
    # slab sl: partitions 0-63 = round 2sl's z-window, 64-127 = round 2sl+1
    mem_d = nc.dram_tensor("mem", [NROUND // 2, 2 * ZWIN, RND_COLS],
                           bf16, kind="ExternalInput")
    out_d = nc.dram_tensor("out", [NROUND // 2, D, 2 * PAIRS_RND], f16,
                           kind="ExternalOutput")

    with TileContext(nc) as tc:
        with ExitStack() as ctx:
            singles = ctx.enter_context(tc.tile_pool(name="singles", bufs=1))
            mempool = ctx.enter_context(tc.tile_pool(name="memp", bufs=3))
            wpool = ctx.enter_context(tc.tile_pool(name="wp", bufs=2))
            kpool = ctx.enter_context(tc.tile_pool(name="kp", bufs=2))
            outpool = ctx.enter_context(tc.tile_pool(name="outp", bufs=2))
            pswpool = ctx.enter_context(tc.tile_pool(name="psw", bufs=2, space="PSUM"))
            psopool = ctx.enter_context(tc.tile_pool(name="pso", bufs=4, space="PSUM"))

            # everything on the sync HW-DGE queue (the scalar queue steals
            # from the same ~190GB/s pipe); compute-critical loads first
            cz = singles.tile([KROWS, ZCOLS + NROUND * CPR], bf16)
            nc.sync.dma_start(out=cz[:], in_=cz_d[:, :])
            zaug = cz[:, 0:ZCOLS]
            coefall = cz[:, ZCOLS:]

            bigmems = []
            for sl in range(NROUND // 2):
                bigmem = mempool.tile([2 * ZWIN, RND_COLS], bf16)
                if sl == NROUND // 2 - 1:
                    # split the last slab so its first groups' mm2 wave
                    # overlaps the trailing half of the transfer
                    hc = RND_COLS // 2
                    nc.sync.dma_start(out=bigmem[:, 0:hc],
                                      in_=mem_d[sl, :, 0:hc])
                    nc.sync.dma_start(out=bigmem[:, hc:],
                                      in_=mem_d[sl, :, hc:])
                else:
                    nc.sync.dma_start(out=bigmem[:], in_=mem_d[sl])
                bigmems.append(bigmem)

            for sl in range(NROUND // 2):
                bigmem = bigmems[sl]
                # both rounds' W columns into one PSUM bank, stacked by
                # partition half (tile_position rows/cols from AP bases)
                psW = pswpool.tile([2 * ZWIN, CPR], f32)
                for h in range(2):
                    rnd = 2 * sl + h
                    zs = FLAV_OF_ROUND[rnd] * ZWIN
                    cs = rnd * CPR
                    nc.tensor.matmul(psW[h * ZWIN:(h + 1) * ZWIN, :],
                                     zaug[:, zs:zs + ZWIN],
                                     coefall[:, cs:cs + CPR],
                                     start=True, stop=True)

                # paired section: W = min(W0, W1); single section: W direct
                wm = wpool.tile([2 * ZWIN, PG * MP], f32)
                pw = psW[:, 0:2 * PG * MP].rearrange("p (jb s) -> p jb s", s=2)
                nc.vector.tensor_reduce(
                    wm[:], pw, axis=mybir.AxisListType.X,
                    op=mybir.AluOpType.min)

                kern = kpool.tile([2 * ZWIN, PAIRS_RND], bf16)
                nc.scalar.activation(kern[:, 0:PG * MP], wm[:],
                                     mybir.ActivationFunctionType.Exp)
                nc.scalar.activation(kern[:, PG * MP:], psW[:, 2 * PG * MP:],
                                     mybir.ActivationFunctionType.Exp)

                outsb = outpool.tile([D, 2 * PAIRS_RND], f16)
                for h in range(2):
                    z0, z1 = h * ZWIN, (h + 1) * ZWIN
                    psO = psopool.tile([D, PAIRS_RND], f32)
                    for g in range(GRP_PER_ROUND):
                        nc.tensor.matmul(psO[:, g * MP:(g + 1) * MP],
                                         bigmem[z0:z1, g * 128:(g + 1) * 128],
                                         kern[z0:z1, g * MP:(g + 1) * MP],
                                         start=True, stop=True)
                    os = h * PAIRS_RND
                    nc.vector.tensor_copy(out=outsb[:, os:os + PAIRS_RND],
                                          in_=psO[:])
                nc.sync.dma_start(out=out_d[sl], in_=outsb[:])

    nc.compile()
    return nc


def _get_nc():
    if "nc" not in _BASS_CACHE:
        _BASS_CACHE["nc"] = _build_nc()
    return _BASS_CACHE["nc"]


def _bf16(x):
    import ml_dtypes
    return x.astype(ml_dtypes.bfloat16)


def _split3(x):
    """f64 -> three bf16 parts summing to ~24 mantissa bits of x."""
    x0 = _bf16(x).astype(np.float64)
    x1 = _bf16(x - x0).astype(np.float64)
    x2 = _bf16(x - x0 - x1).astype(np.float64)
    return x0, x1, x2


def _host_coeffs(ray_origin, ray_dir):
    """Quadratic coefficients of W0/W1 in u = z-64, in f64."""
    o = ray_origin.astype(np.float64)
    d = ray_dir.astype(np.float64)
    d2 = (d * d).sum(-1)
    kap = 2.0 - d2
    od = (o * d).sum(-1)
    g = np.arange(D, dtype=np.float64)
    gxy_x = np.repeat(g, D)
    gxy_y = np.tile(g, D)
    c1 = 1.0 / (2 * SIGMA ** 2)
    c3 = 1.0 / TAU
    alpha = gxy_x[:, None] * d[None, :, 0] + gxy_y[:, None] * d[None, :, 1] - od[None, :]
    t64 = 64.0 * d[None, :, 2] + alpha                      # [NCHUNK, B]
    e = 64.0 - o[:, 2]                                      # [B]
    gamma = (gxy_x[:, None] - o[None, :, 0]) ** 2 + (gxy_y[:, None] - o[None, :, 1]) ** 2
    A0 = np.broadcast_to((-c1 + c1 * kap * d[:, 2] ** 2)[None, :], t64.shape)
    B0 = -2 * c1 * e[None, :] + 2 * c1 * kap[None, :] * d[None, :, 2] * t64
    C0 = -c1 * (gamma + e[None, :] ** 2) + c1 * kap[None, :] * t64 ** 2
    B1 = B0 - c3 * d[None, :, 2]
    C1 = C0 - c3 * t64
    return A0, B0, C0, B1, C1


def _pack_cols(Aq, Bq, Cq):
    """[...] f64 quadratic -> [11, ...] bf16 split rows.
    Row order: [C0,B0,Ah0,Al0, C1,B1,Ah1,Al1, C2,B2,Ah2]."""
    C_0, C_1, C_2 = _split3(Cq)
    B_0, B_1, B_2 = _split3(Bq)
    A_0, A_1, A_2 = _split3(Aq)
    rows = [C_0, B_0, A_0, A_0, C_1, B_1, A_1, A_1, C_2, B_2, A_2]
    return np.stack([_bf16(r) for r in rows])


def _zaug_rows():
    """Per-flavor basis columns: [11, NFLAV * ZWIN] bf16."""
    cols = []
    for f in range(NFLAV):
        z0 = f * 32
        u = np.arange(z0, z0 + ZWIN, dtype=np.float64) - 64.0
        u2 = u * u
        uh = _bf16(u2).astype(np.float64)
        ul = u2 - uh
        one = np.ones_like(u)
        rows = [one, u, uh, ul, one, u, uh, ul, one, u, uh]
        cols.append(np.stack([_bf16(r) for r in rows]))
    return np.concatenate(cols, axis=1)


def _select_pairs(A0, B0, C0, B1, C1):
    """Kept (chunk, ray) pairs with exact z-sums >= S_THRESH and their
    z-supports [lo, hi). Returns (chunk, ray, lo, hi) sorted by chunk."""
    u = np.arange(D, dtype=np.float64) - 64.0

    def grid_max(Bq, Cq):
        us = np.round(np.clip(-Bq / (2 * A0), u[0], u[-1]))
        best = A0 * us * us + Bq * us + Cq
        for dd in (-1.0, 1.0):
            u2 = np.clip(us + dd, u[0], u[-1])
            best = np.maximum(best, A0 * u2 * u2 + Bq * u2 + Cq)
        return best

    wmax = np.minimum(grid_max(B0, C0), grid_max(B1, C1))
    kmax = np.exp(np.maximum(wmax, -745.0))
    # S <= 128 * kmax, so this candidate set provably covers {S >= thresh}
    ci, ri = np.nonzero(kmax >= S_THRESH / 256.0)
    a, b0, c0 = A0[ci, ri, None], B0[ci, ri, None], C0[ci, ri, None]
    b1, c1 = B1[ci, ri, None], C1[ci, ri, None]
    W = np.minimum(a * u * u + b0 * u + c0, a * u * u + b1 * u + c1)
    kern = np.exp(np.maximum(W, -745.0))
    S = kern.sum(-1)
    keep = S >= S_THRESH
    ci, ri, kern = ci[keep], ri[keep], kern[keep]
    csum = np.cumsum(kern, -1)
    tot = csum[:, -1:]
    lo = (csum < ZEPS).sum(-1)
    hi = D - ((tot - csum) < ZEPS).sum(-1)
    hi = np.maximum(hi, lo + 1)
    # clamp ultra-wide supports (none expected) to their heaviest window
    wide = (hi - lo) > ZWIN
    if wide.any():
        zc = np.argmax(kern[wide], -1)
        lo[wide] = np.clip(zc - ZWIN // 2, 0, D - ZWIN)
        hi[wide] = lo[wide] + ZWIN
    # branch type: 0 = W0 alone suffices, 1 = W1 alone, 2 = straddler
    zz = np.arange(D)[None, :]
    msk = (zz >= lo[:, None]) & (zz < hi[:, None])
    a, b0, c0 = A0[ci, ri, None], B0[ci, ri, None], C0[ci, ri, None]
    b1, c1 = B1[ci, ri, None], C1[ci, ri, None]
    S0 = (np.exp(np.maximum(a * u * u + b0 * u + c0, -745.0)) * msk).sum(-1)
    S1 = (np.exp(np.maximum(a * u * u + b1 * u + c1, -745.0)) * msk).sum(-1)
    Sm = (kern * msk).sum(-1)
    bt = np.full(len(ci), 2, np.int64)
    bt[(S1 - Sm) <= BTOL] = 1
    bt[(S0 - Sm) <= BTOL] = 0
    return ci, ri, lo, hi, bt


def _flavors_of(lo, hi):
    """Bitmask of window flavors whose [32f, 32f+ZWIN) covers [lo, hi)."""
    m = 0
    for f in range(NFLAV):
        z0 = 32 * f
        if z0 <= lo and hi <= z0 + ZWIN:
            m |= 1 << f
    return m


def _pack_core(items):
    """items: list of (chunk, [(ray, lo, hi), ...]). Split per chunk into
    flavor-subsets (keeping each subset's full allowed-flavor mask), assign
    subsets to flavors balancing chunk-slot load, then bin-pack each flavor
    into groups of <= SLOTS chunks / <= MP pairs laid out into its fixed
    rounds. Returns (slot_chunk [NG, SLOTS], pr_slot/pr_ray/pr_valid)."""
    subsets = []
    for chunk, rays in items:
        rem = [((r, bt), _flavors_of(lo, hi)) for (r, lo, hi, bt) in rays]
        while rem:
            # pick the flavor covering the most remaining rays
            cnt = [sum(1 for _, m in rem if m >> f & 1) for f in range(NFLAV)]
            f = int(np.argmax(cnt))
            taken = [(rb, m) for rb, m in rem if m >> f & 1]
            rem = [(rb, m) for rb, m in rem if not (m >> f & 1)]
            mk = (1 << NFLAV) - 1
            for _, m in taken:
                mk &= m
            subsets.append((chunk, [rb for rb, _ in taken], mk))
    # assign inflexible subsets first, then flexible ones to the lightest
    # flavor (chunk-slots are the binding capacity)
    subsets.sort(key=lambda s: bin(s[2]).count("1"))
    loads = [0] * NFLAV
    flav_items = [[] for _ in range(NFLAV)]
    for chunk, rays, mk in subsets:
        cand = [f for f in range(NFLAV) if mk >> f & 1]
        f = min(cand, key=lambda f: loads[f])
        flav_items[f].append((chunk, rays))
        loads[f] += 1

    slot_chunk = np.full((NG, SLOTS), -1, np.int64)
    pr_slot = np.zeros((NG, MP), np.int64)
    pr_ray = np.zeros((NG, MP), np.int64)
    pr_btype = np.zeros((NG, MP), np.int64)
    pr_valid = np.zeros((NG, MP), bool)
    rounds_of_flav = [[r for r in range(NROUND) if FLAV_OF_ROUND[r] == f]
                      for f in range(NFLAV)]
    for f in range(NFLAV):
        # paired group slots first; straddler-containing items lead so
        # every straddler pair lands in a paired group
        slots_avail = (
            [r * GRP_PER_ROUND + j for r in rounds_of_flav[f]
             for j in range(PG)] +
            [r * GRP_PER_ROUND + j for r in rounds_of_flav[f]
             for j in range(PG, GRP_PER_ROUND)])
        fitems = sorted(flav_items[f],
                        key=lambda it: -max(bt for _, bt in it[1]))
        gi = 0
        g = slots_avail[0]
        ns = 0
        npair = 0
        for chunk, rays in fitems:
            while len(rays) > 0:
                take = rays[:MP]
                if ns >= SLOTS or npair + len(take) > MP:
                    gi += 1
                    if gi >= len(slots_avail):
                        raise RuntimeError(
                            "group capacity exceeded; raise S_THRESH")
                    g = slots_avail[gi]
                    ns = 0
                    npair = 0
                slot_chunk[g, ns] = chunk
                for r, bt in take:
                    if bt == 2 and g % GRP_PER_ROUND >= PG:
                        raise RuntimeError("straddler outside paired group")
                    pr_slot[g, npair] = ns
                    pr_ray[g, npair] = r
                    pr_btype[g, npair] = bt
                    pr_valid[g, npair] = True
                    npair += 1
                ns += 1
                rays = rays[MP:]
    return slot_chunk, pr_slot, pr_ray, pr_btype, pr_valid


def _prep_inputs(ray_origin, ray_dir, memory):
    import ml_dtypes
    A0, B0, C0, B1, C1 = _host_coeffs(ray_origin, ray_dir)
    w0 = _pack_cols(A0, B0, C0)          # [11, NCHUNK, B]
    w1 = _pack_cols(A0, B1, C1)
    zaug = _zaug_rows()

    ci, ri, lo, hi, bt = _select_pairs(A0, B0, C0, B1, C1)
    order = np.argsort(ci, kind="stable")
    ci, ri, lo, hi, bt = ci[order], ri[order], lo[order], hi[order], bt[order]
    uchunks, starts = np.unique(ci, return_index=True)
    starts = list(starts) + [len(ci)]
    items = [(uchunks[j],
              [(ri[t], lo[t], hi[t], bt[t])
               for t in range(starts[j], starts[j + 1])])
             for j in range(len(uchunks))]

    mem = np.ascontiguousarray(memory, dtype=np.float32).reshape(NCHUNK, D, C)
    mem_bf = mem.astype(ml_dtypes.bfloat16)
    in_maps = []
    extract = []
    CPR = PG * MP * 2 + (GRP_PER_ROUND - PG) * MP
    for k in range(NCORES):
        slot_chunk, pr_slot, pr_ray, pr_btype, pr_valid = _pack_core(
            items[k::NCORES])
        safe = np.maximum(slot_chunk, 0)
        # z-windowed mem gather: group g uses window of its round's flavor
        z0g = np.array([32 * FLAV_OF_ROUND[g // GRP_PER_ROUND]
                        for g in range(NG)])
        zidx = z0g[:, None] + np.arange(ZWIN)[None, :]       # [NG, ZWIN]
        mk = mem_bf[safe[:, :, None], zidx[:, None, :]]      # [NG, SLOTS, ZWIN, C]
        mk[slot_chunk < 0] = 0
        # [NROUND // 2, 2 * ZWIN, GRP_PER_ROUND * SLOTS * C]: two-round
        # slabs stacked along the partition axis
        mk = np.ascontiguousarray(
            mk.reshape(NROUND // 2, 2, GRP_PER_ROUND, SLOTS, ZWIN, C)
            .transpose(0, 1, 4, 2, 3, 5)).reshape(NROUND // 2, 2 * ZWIN, -1)

        # coef columns: paired groups get (W0, W1) per pair (clean pairs
        # duplicate their active branch); single groups get one branch
        pc = np.maximum(slot_chunk, 0)[
            np.arange(NG)[:, None], np.maximum(pr_slot, 0)]   # chunk per pair
        w0c = w0[:, pc, pr_ray]                    # [11, NG, MP]
        w1c = w1[:, pc, pr_ray]
        wb = np.where((pr_btype == 1)[None], w1c, w0c)
        pa = np.where((pr_btype == 2)[None], w0c, wb)
        pb = np.where((pr_btype == 2)[None], w1c, wb)
        for arr in (wb, pa, pb):
            arr[:, ~pr_valid] = 0
            arr[0, ~pr_valid] = -30000.0           # kern = exp(-30000) = 0
        gidx = np.arange(NG).reshape(NROUND, GRP_PER_ROUND)
        paired = np.stack([pa[:, gidx[:, :PG]], pb[:, gidx[:, :PG]]],
                          axis=-1).reshape(KROWS, NROUND, PG * MP * 2)
        single = wb[:, gidx[:, PG:]].reshape(KROWS, NROUND, -1)
        ck = np.concatenate([paired, single], axis=2).reshape(KROWS, -1)
        cz = np.ascontiguousarray(np.concatenate([zaug, ck], axis=1))
        in_maps.append({"cz": cz, "mem": mk})
        extract.append((pr_slot, pr_ray, pr_valid))
    return in_maps, extract


def _extract(results, extract):
    out = np.zeros((B, C), np.float64)
    for res, (pr_slot, pr_ray, pr_valid) in zip(results, extract):
        blocks = res["out"].astype(np.float64)  # [NROUND//2, 128, 2*PAIRS_RND]
        arr = blocks.reshape(NROUND // 2, D, 2, GRP_PER_ROUND, MP)
        arr = arr.transpose(0, 2, 3, 1, 4).reshape(NG, D, MP)
        # val[g, p, c] = arr[g, 16 * pr_slot[g, p] + c, p]
        arrT = arr.transpose(0, 2, 1)              # [NG, MP, 128]
        row = (C * pr_slot)[:, :, None] + np.arange(C)[None, None, :]
        val = np.take_along_axis(arrT, row, axis=2)   # [NG, MP, C]
        np.add.at(out, pr_ray[pr_valid], val[pr_valid])
    return np.ascontiguousarray(out).astype(np.float32)   # [B, C]


def run_kernel(ray_origin, ray_dir, memory, trace=False, **run_kwargs):
    """Run on 8 NeuronCores; returns ([B,C] output, BassKernelResults)."""
    from concourse.bass_utils import run_bass_kernel_spmd
    nc = _get_nc()
    in_maps, extract = _prep_inputs(ray_origin, ray_dir, memory)
    br = run_bass_kernel_spmd(nc, in_maps, core_ids=list(range(NCORES)),
                              trace=trace, **run_kwargs)
    return _extract(br.results, extract), br


def kernel(ray_origin, ray_dir, memory):
    out, _ = run_kernel(np.asarray(ray_origin), np.asarray(ray_dir),
                        np.asarray(memory))
    return out
